# revision 22
# baseline (speedup 1.0000x reference)
"""Multi-head attention kernel for Trainium2 (Bass/Tile), 8 NeuronCores.

Problem: nn_MultiHeadAttention  (B=4, S=2048, D=1024, H=16, DK=64)
    out = softmax((q Wq^T + bq)(k Wk^T + bk)^T / sqrt(DK)) (v Wv^T + bv) Wo^T + bo

Sharding: core c = 2*b + g handles batch b and head-group g (8 heads = 512
features).  Each core computes its batch's attention for its heads plus a
partial output projection; the host sums the two partials per batch.

Math simplifications done on the host (exact):
  - k-bias bk drops out (softmax shift invariance along keys).
  - v-bias bv folds into an effective output bias bo_eff = bo + Wo @ bv.
  - the 1/sqrt(DK) logit scale is folded into Wq/bq.

Schedule: dense Q'/K' projection phase, then per-head attention with the
PV matmuls software-pipelined one key-chunk behind the S^T/exp stream
(for si: S^T(si) -> exp(si) -> PV(si-1)), so the PE never sits in the
serial exp->PV->S^T dependency chain and the ScalarE exp stream (~275us,
the irreducible softmax cost) runs at ~90% occupancy through the
attention phase.  Head 0's attention is fused into the V-projection
loop; the output projection runs as a dense tail.
PSUM: S^T double-buffer 2x[128,1024] (4 banks) + 4 PV accumulators
[65,512] (4 banks) = 8 banks.
"""

import numpy as np
import ml_dtypes
from contextlib import ExitStack

import concourse.bass as bass
import concourse.tile as tile
from concourse import bacc, mybir
from concourse.bass import ts, ds
from concourse.bass_utils import run_bass_kernel_spmd

B, S, D, H, DK = 4, 2048, 1024, 16, 64
N_CORES = 8
F32 = mybir.dt.float32
BF16 = mybir.dt.bfloat16
AF = mybir.ActivationFunctionType
ALU = mybir.AluOpType
BF16NP = ml_dtypes.bfloat16


def build_nc(s: int = S):
    """Build + compile the per-core Bass module (SPMD: same NEFF, per-core data)."""
    assert s % 512 == 0
    nsi = s // 128   # 128-row key chunks
    nf = s // 512    # 512-col query chunks

    nc = bacc.Bacc("TRN2", target_bir_lowering=False, debug=False)

    qT = nc.dram_tensor("qT", [D, s], BF16, kind="ExternalInput").ap()
    kT = nc.dram_tensor("kT", [D, s], BF16, kind="ExternalInput").ap()
    vT = nc.dram_tensor("vT", [D, s], BF16, kind="ExternalInput").ap()
    wq = nc.dram_tensor("wq", [D, 512], BF16, kind="ExternalInput").ap()
    wk = nc.dram_tensor("wk", [D, 512], BF16, kind="ExternalInput").ap()
    wv = nc.dram_tensor("wv", [D, 512], BF16, kind="ExternalInput").ap()
    wo = nc.dram_tensor("wo", [512, D], BF16, kind="ExternalInput").ap()
    bq = nc.dram_tensor("bq", [128, 4], F32, kind="ExternalInput").ap()
    outT = nc.dram_tensor("outT", [D, s], F32, kind="ExternalOutput").ap()

    with tile.TileContext(nc) as tc, ExitStack() as ctx:
        pers = ctx.enter_context(tc.tile_pool(name="pers", bufs=1))
        pspool = ctx.enter_context(tc.tile_pool(name="ps", bufs=4, space="PSUM"))
        pss = ctx.enter_context(tc.tile_pool(name="pss", bufs=2, space="PSUM"))
        epool = ctx.enter_context(tc.tile_pool(name="e", bufs=38))

        QT = pers.tile([128, 4, s], BF16)       # Q'^T  [feature, seq]
        KT = pers.tile([128, 4, s], BF16)       # K^T   [feature, seq]
        V = pers.tile([128, nsi, 8, 66], BF16)  # V nat [seq, head, dv|ones|pad]
        O = pers.tile([128, 4, s], BF16)        # O^T normalized
        WO = pers.tile([128, 4, D], BF16)
        BQ = pers.tile([128, 4], F32)

        nc.sync.dma_start(WO[:], wo.rearrange("(o p) e -> p o e", p=128))
        nc.sync.dma_start(BQ[:], bq)
        nc.vector.memset(V[:, :, :, 64:65], 1.0)

        hw_ = min(1024, s)      # S^T psum tile width (2 PSUM banks fp32)
        fph = hw_ // 512        # 512-col f-chunks per psum tile

        ph1 = ExitStack()
        xpool = ph1.enter_context(tc.tile_pool(name="x", bufs=16))
        wpool = ph1.enter_context(tc.tile_pool(name="w", bufs=1))

        def st_pair(g, qh, si):
            sta = pss.tile([128, 1024], F32, tag="s", name=f"sa_{g}_{qh}_{si}")
            stb = pss.tile([128, 1024], F32, tag="s", name=f"sb_{g}_{qh}_{si}")
            for fo in range(2):
                f = 2 * qh + fo
                nc.tensor.matmul(
                    sta[:, ts(fo, 512)],
                    lhsT=KT[ds(0, 64), g, ts(si, 128)],
                    rhs=QT[ds(0, 64), g, ts(f, 512)],
                    start=True, stop=True,
                )
                nc.tensor.matmul(
                    stb[:, ts(fo, 512)],
                    lhsT=KT[ds(64, 64), g, ts(si, 128)],
                    rhs=QT[ds(64, 64), g, ts(f, 512)],
                    start=True, stop=True,
                )
            ea = epool.tile([128, 1024], BF16, tag="e", name=f"ea_{g}_{qh}_{si}")
            eb = epool.tile([128, 1024], BF16, tag="e", name=f"eb_{g}_{qh}_{si}")
            nc.scalar.activation(ea[:], sta[:], AF.Exp)
            nc.scalar.activation(eb[:], stb[:], AF.Exp)
            return (ea, eb)

        def pv_pair(g, es, pos, si):
            ea, eb = es[si]
            st_, sp = (si == 0), (si == nsi - 1)
            for fo in range(2):
                nc.tensor.matmul(
                    pos[fo][0:65, :], lhsT=V[:, si, 2 * g, 0:65],
                    rhs=ea[:, ts(fo, 512)], start=st_, stop=sp)
            for fo in range(2):
                nc.tensor.matmul(
                    pos[2 + fo][0:65, :], lhsT=V[:, si, 2 * g + 1, 0:65],
                    rhs=eb[:, ts(fo, 512)], start=st_, stop=sp)

        def pv_finish(g, qh, pos):
            for hh, pbase in ((0, 0), (64, 2)):
                ou = oupool.tile([65, 1024], F32, tag=f"ou{hh}",
                                 name=f"ou_{g}_{qh}_{hh}")
                for fo in range(2):
                    nc.vector.tensor_copy(
                        ou[:, ts(fo, 512)], pos[pbase + fo][0:65, :])
                rr = bpool.tile([1, 1024], BF16, tag="rr",
                                name=f"rr_{g}_{qh}_{hh}")
                with nc.allow_low_precision(reason="denom recip bf16"):
                    nc.vector.reciprocal(rr[:], ou[ds(64, 1), :])
                dscr = dpool.tile([1, 1024], BF16, tag="dscr",
                                  name=f"dscr_{g}_{qh}_{hh}")
                nc.sync.dma_start(dscr[:], rr[:])
                bsb = bpool.tile([64, 1024], BF16, tag="bsb",
                                 name=f"bsb_{g}_{qh}_{hh}")
                nc.sync.dma_start(bsb[:], dscr[:].to_broadcast((64, 1024)))
                nc.vector.tensor_tensor(
                    O[ds(hh, 64), g, ds(qh * 1024, 1024)],
                    ou[0:64, :],
                    bsb[0:64, :],
                    ALU.mult,
                )

        # ---- phase 1: Q'/K projections --------------------------------
        for xdram, wdram, dst, bias in ((qT, wq, QT, BQ), (kT, wk, KT, None)):
            wt = wpool.tile([128, 8, 512], BF16, tag="w")
            nc.sync.dma_start(wt[:], wdram.rearrange("(o p) m -> p o m", p=128))
            for f in range(nf):
                xts = []
                for ki in range(8):
                    xt = xpool.tile([128, 512], BF16, tag="x")
                    nc.sync.dma_start(
                        xt[:], xdram[ds(ki * 128, 128), ds(f * 512, 512)]
                    )
                    xts.append(xt)
                for pc in range(4):
                    ps = pspool.tile([128, 512], F32, tag="ps")
                    for ki in range(8):
                        nc.tensor.matmul(
                            ps[:],
                            lhsT=wt[:, ki, ts(pc, 128)],
                            rhs=xts[ki][:],
                            start=(ki == 0),
                            stop=(ki == 7),
                        )
                    if bias is not None:
                        nc.vector.tensor_scalar_add(
                            dst[:, pc, ts(f, 512)], ps[:], bias[:, pc : pc + 1]
                        )
                    else:
                        nc.vector.tensor_copy(dst[:, pc, ts(f, 512)], ps[:])

        ph2b = ExitStack()
        bpool = ph2b.enter_context(tc.tile_pool(name="b", bufs=2))
        oupool = ph2b.enter_context(tc.tile_pool(name="ou", bufs=2))
        dpool = ph2b.enter_context(tc.tile_pool(name="dscr", bufs=2, space="DRAM"))

        es0 = []
        pos0 = [
            pspool.tile([128, 512], F32, tag="ps", name=f"pos0_{i}")
            for i in range(4)
        ]
        wt = wpool.tile([128, 8, 512], BF16, tag="w")
        nc.sync.dma_start(wt[:], wv.rearrange("(o p) m -> p o m", p=128))
        for f in range(nf):
            xts = []
            for ki in range(8):
                xt = xpool.tile([128, 512], BF16, tag="x")
                nc.sync.dma_start(xt[:], vT[ds(ki * 128, 128), ds(f * 512, 512)])
                xts.append(xt)
            for sj in range(4):
                si = f * 4 + sj
                vps = pss.tile([128, 1024], F32, tag="s", name=f"vps_{si}")
                for ki in range(8):
                    nc.tensor.matmul(
                        vps[:, 0:512],
                        lhsT=xts[ki][:, ts(sj, 128)],
                        rhs=wt[:, ki, :],
                        start=(ki == 0),
                        stop=(ki == 7),
                    )
                nc.vector.tensor_copy(
                    V[:, si, :, 0:64],
                    vps[:, 0:512].rearrange("p (h d) -> p h d", h=8),
                )
                es0.append(st_pair(0, 0, si))
                if si >= 1:
                    pv_pair(0, es0, pos0, si - 1)
        pv_pair(0, es0, pos0, nsi - 1)
        pv_finish(0, 0, pos0)

        for g, qh in ((1, 0), (2, 0), (3, 0), (0, 1), (1, 1), (2, 1), (3, 1)):
            es = []
            pos = [
                pspool.tile([128, 512], F32, tag="ps", name=f"pos_{g}_{qh}_{i}")
                for i in range(4)
            ]
            for si in range(nsi):
                es.append(st_pair(g, qh, si))
                if si >= 1:
                    pv_pair(g, es, pos, si - 1)
            pv_pair(g, es, pos, nsi - 1)
            pv_finish(g, qh, pos)
        ph2b.close()
        ph1.close()

        # ---- phase 3: output projection (partial over this core's heads)
        opool = ctx.enter_context(tc.tile_pool(name="ostage", bufs=3))
        outr = outT.rearrange("(o p) n -> p o n", p=128)
        for pe in range(8):
            for f in range(nf):
                ps = pspool.tile([128, 512], F32, tag="ps")
                for ki in range(4):
                    nc.tensor.matmul(
                        ps[:],
                        lhsT=WO[:, ki, ts(pe, 128)],
                        rhs=O[:, ki, ts(f, 512)],
                        start=(ki == 0),
                        stop=(ki == 3),
                    )
                ot = opool.tile([128, 512], F32, tag="ot")
                nc.vector.tensor_copy(ot[:], ps[:])
                nc.sync.dma_start(outr[:, pe, ts(f, 512)], ot[:])

    nc.compile()
    return nc


_NC_CACHE: dict = {}


def get_nc(s: int = S):
    if s not in _NC_CACHE:
        _NC_CACHE[s] = build_nc(s)
    return _NC_CACHE[s]


def _prep_in_maps(q, k, v, Wq, bq, Wk, Wv, Wo):
    """Host-side shard prep: per-core input dicts (cheap numpy reshapes)."""
    f32 = np.float32
    scale = 1.0 / np.sqrt(DK)
    xT = {}
    for b in range(B):
        xT[b] = (
            np.ascontiguousarray(q[b].T).astype(BF16NP),
            np.ascontiguousarray(k[b].T).astype(BF16NP),
            np.ascontiguousarray(v[b].T).astype(BF16NP),
        )
    per_g = {}
    for g in range(2):
        F = slice(512 * g, 512 * g + 512)
        per_g[g] = dict(
            wq=np.ascontiguousarray(Wq[F].T * scale).astype(BF16NP),
            wk=np.ascontiguousarray(Wk[F].T).astype(BF16NP),
            wv=np.ascontiguousarray(Wv[F].T).astype(BF16NP),
            wo=np.ascontiguousarray(Wo[:, F].T).astype(BF16NP),
            bq=np.ascontiguousarray(
                (bq[F] * scale).reshape(4, 128).T, dtype=f32
            ),
        )
    in_maps = []
    for c in range(N_CORES):
        b, g = c // 2, c % 2
        qb, kb, vb = xT[b]
        in_maps.append(dict(qT=qb, kT=kb, vT=vb, **per_g[g]))
    return in_maps


def kernel(q, k, v, Wq, bq, Wk, bk, Wv, bv, Wo, bo):
    q, k, v = (np.asarray(x, np.float32) for x in (q, k, v))
    Wq, bq, Wk, bk = (np.asarray(x, np.float32) for x in (Wq, bq, Wk, bk))
    Wv, bv, Wo, bo = (np.asarray(x, np.float32) for x in (Wv, bv, Wo, bo))

    nc = get_nc(S)
    in_maps = _prep_in_maps(q, k, v, Wq, bq, Wk, Wv, Wo)
    res = run_bass_kernel_spmd(nc, in_maps, core_ids=list(range(N_CORES)))

    # bk drops out of softmax; bv folds into an effective output bias.
    bo_eff = (
        bo.astype(np.float64) + Wo.astype(np.float64) @ bv.astype(np.float64)
    ).astype(np.float32)
    out = np.empty((B, S, D), np.float32)
    for b in range(B):
        acc = res.results[2 * b]["outT"] + res.results[2 * b + 1]["outT"]
        out[b] = acc.T + bo_eff
    return out
